# revision 18
# baseline (speedup 1.0000x reference)
"""Trainium2 Bass kernel for nn_Attention (dense_transformer).

Math (per fused-batch element, 32 total = b*m):
    qkv = x @ w_qkv ; split q,k,v into 8 heads of 64
    sim = (q/8) @ k^T  (+ pos_bias term that is constant along the softmax
                        axis -> provably no effect on softmax output, dropped)
    attn = softmax(sim); out = (attn @ v) heads-concat @ w_out

Sharding: pure data-parallel over the fused (b*m)=32 axis -> 4 elements
per core on 8 cores, no collectives. Weights replicated.

Kernel strategy (per core, all-transposed dataflow, bf16 matmuls):
    xT   = PE-transpose(x)                        [c, n]
    qT,kT (pair-stacked) = W_qk^T @ xT            [e_slice, n]  (psum f32)
    V    = xT-slices @ W_v                        [n, e_v] natural layout,
           stored interleaved [n, h, 65] with a ones column per head
    S^T  = kT_h^T-slice @ qT_h                    [j, i] per head
    P^T  = exp(0.125 * S^T)  (no max subtraction: |logits| <= ~8)
    outT_h (rows 0..63) + L_h (row 64) = V1_h^T @ P^T   (ones-column trick)
    OT   = outT_h * broadcast(1/L_h)  (K=1 matmul broadcast + DVE mul)
    out  = OT-slices^T @ w_out        [n, c] -> DMA out
"""

import os
import sys

for _p in ("/root/.axon_site/_ro/trn_rl_repo", "/opt/trn_rl_repo"):
    if os.path.isdir(_p) and _p not in sys.path:
        sys.path.append(_p)

import numpy as np

# ---- problem constants (hardcoded per spec) ----
B, M, N, C = 4, 8, 512, 512
HEADS, DHEAD = 8, 64
E3 = 3 * 512
NCORES = 8
BPC = (B * M) // NCORES  # batch elements per core = 4
BCAST_MODE = "gpsimd"  # "gpsimd" | "pe"
RECIP_MODE = "approx_sbuf"  # "exact" | "approx_sbuf" | "approx_psum"

_cache = {}


def _build():
    import concourse.bass as bass
    import concourse.mybir as mybir
    import concourse.tile as tile
    from concourse import bacc
    from concourse.masks import make_identity

    f32 = mybir.dt.float32
    bf16 = mybir.dt.bfloat16
    f32r = mybir.dt.float32r
    EXP = mybir.ActivationFunctionType.Exp

    nc = bacc.Bacc("TRN2", target_bir_lowering=False, debug=False,
                   num_devices=NCORES)

    x_ext = nc.declare_dram_parameter("x", [BPC, N, C], f32, isOutput=False)
    wq_ext = nc.declare_dram_parameter("w_qkv", [C, E3], f32, isOutput=False)
    wo_ext = nc.declare_dram_parameter("w_out", [512, 512], f32, isOutput=False)
    out_ext = nc.declare_dram_parameter("out", [BPC, N, C], f32, isOutput=True)

    from contextlib import ExitStack

    with tile.TileContext(nc) as tc, ExitStack() as ctx:
        # ---------------- pools ----------------
        p_const = ctx.enter_context(tc.tile_pool(name="const", bufs=1))
        p_stage = ctx.enter_context(tc.tile_pool(name="stage", bufs=1))
        p_x = ctx.enter_context(tc.tile_pool(name="x", bufs=2))
        p_xT = ctx.enter_context(tc.tile_pool(name="xT", bufs=2))
        p_qk = ctx.enter_context(tc.tile_pool(name="qk", bufs=2))
        p_v = ctx.enter_context(tc.tile_pool(name="v", bufs=2))
        p_pt = ctx.enter_context(tc.tile_pool(name="pt", bufs=2))
        p_oT = ctx.enter_context(tc.tile_pool(name="oT", bufs=2))
        p_out = ctx.enter_context(tc.tile_pool(name="out", bufs=2))
        p_small = ctx.enter_context(tc.tile_pool(name="small", bufs=4))

        # tr and st share one 4-deep pool (same tag) so STs can run 4 ahead
        # of the ACT exps; 4 + 2 + 2 = 8 PSUM banks exactly
        ps_st = ctx.enter_context(tc.tile_pool(name="ps_st", bufs=4, space="PSUM"))
        ps_tr = ps_st
        ps_proj = ctx.enter_context(tc.tile_pool(name="ps_proj", bufs=2, space="PSUM"))
        ps_ot = ctx.enter_context(tc.tile_pool(name="ps_ot", bufs=2, space="PSUM"))
        if BCAST_MODE != "gpsimd":
            ps_bc = ctx.enter_context(
                tc.tile_pool(name="ps_bc", bufs=1, space="PSUM"))

        # ---------------- constants ----------------
        ident = p_const.tile([128, 128], bf16)
        make_identity(nc, ident[:])
        ones_bc = p_const.tile([1, 64], bf16)
        nc.vector.memset(ones_bc[:], 1.0)

        # weights: fast HWDGE f32 DMA into staging, DVE cast to bf16
        wq_stage = p_stage.tile([128, 4, E3], f32, tag="wqs")
        nc.sync.dma_start(out=wq_stage[:],
                          in_=wq_ext.ap().rearrange("(ct p) e -> p ct e", p=128))
        wq_sb = p_const.tile([128, 4, E3], bf16)
        nc.vector.tensor_copy(wq_sb[:], wq_stage[:])

        wo_stage = p_stage.tile([128, 4, 512], f32, tag="wos")
        nc.sync.dma_start(out=wo_stage[:],
                          in_=wo_ext.ap().rearrange("(t p) c -> p t c", p=128))
        wo_sb = p_const.tile([128, 4, 512], bf16)
        nc.vector.tensor_copy(wo_sb[:], wo_stage[:])

        # ---------------- per batch element ----------------
        for b in range(BPC):
            # x [512, 512] f32 -> SBUF bf16 [128, nt=4, 512] (SWDGE cast DMA
            # on otherwise-idle gpsimd queues; keeps the PE all-bf16 so FWL
            # weight loads stay enabled)
            x_sb = p_x.tile([128, 4, C], bf16, tag="x")
            nc.gpsimd.dma_start(out=x_sb[:],
                                in_=x_ext[b].rearrange("(nt p) c -> p nt c", p=128))

            # xT bf16 [128(c), ct=4, 512(n)] via PE transposes; 4 transposes
            # share one psum bank so one DVE copy drains all of them
            xT = p_xT.tile([128, 4, N], bf16, tag="xT")
            for ct in range(4):
                tr_ps = ps_tr.tile([128, 512], bf16, tag="st", name="tr_ps")
                for nt in range(4):
                    nc.tensor.transpose(
                        tr_ps[:, nt * 128:(nt + 1) * 128],
                        x_sb[:, nt, ct * 128:(ct + 1) * 128], ident[:])
                nc.vector.tensor_copy(xT[:, ct, :], tr_ps[:])

            # q/k projections, pair-stacked: slice s covers e in [s*128,(s+1)*128)
            # s=0..3 -> q head pairs, s=4..7 -> k head pairs
            qkT = p_qk.tile([128, 8, N], bf16, tag="qkT")
            for s in range(8):
                pr_ps = ps_proj.tile([128, N], f32, tag="proj")
                for ct in range(4):
                    nc.tensor.matmul(
                        pr_ps[:],
                        wq_sb[:, ct, s * 128:(s + 1) * 128],
                        xT[:, ct, :],
                        start=(ct == 0), stop=(ct == 3))
                nc.vector.tensor_copy(qkT[:, s, :], pr_ps[:])

            # V natural layout [n, h, d] + ones column: [128, nt=4, 8, 65]
            v_sb = p_v.tile([128, 4, 8, 65], bf16, tag="v")
            nc.vector.memset(v_sb[:, :, :, 64:65], 1.0)
            for nt in range(4):
                pv_ps = ps_proj.tile([128, N], f32, tag="proj")
                for ct in range(4):
                    nc.tensor.matmul(
                        pv_ps[:],
                        xT[:, ct, nt * 128:(nt + 1) * 128],
                        wq_sb[:, ct, 1024:1536],
                        start=(ct == 0), stop=(ct == 3))
                nc.vector.tensor_copy(
                    v_sb[:, nt, :, 0:64],
                    pv_ps[:].rearrange("p (h d) -> p h d", d=64))

            # attention, head pairs stacked on partitions 0-63 / 64-127.
            # Software-pipelined one pair ahead: STs/exps of pair p+1 issue
            # before the PVs of pair p, so the PE FIFO never head-of-line
            # blocks waiting for ACT exp results.
            oT = p_oT.tile([128, 4, N], bf16, tag="oT")
            pts_by_pair = {}

            def emit_st(pair):
                pts = [p_pt.tile([128, 4, N], bf16, tag=f"pt{sub}",
                                 name=f"pt{sub}")
                       for sub in range(2)]
                pts_by_pair[pair] = pts
                for jt in range(4):
                    for sub in range(2):
                        lo, hi = sub * 64, (sub + 1) * 64
                        st_ps = ps_st.tile([128, N], f32, tag="st",
                                           name="st_ps")
                        nc.tensor.matmul(
                            st_ps[:],
                            qkT[lo:hi, 4 + pair, jt * 128:(jt + 1) * 128],
                            qkT[lo:hi, pair, :],
                            start=True, stop=True)
                        nc.scalar.activation(
                            pts[sub][:, jt, :], st_ps[:], EXP,
                            scale=float(DHEAD) ** -0.5)

            def emit_pv(pair):
                pts = pts_by_pair.pop(pair)
                for sub in range(2):
                    h = 2 * pair + sub
                    ot_ps = ps_ot.tile([128, N], f32, tag="ot", name="ot_ps")
                    for jt in range(4):
                        nc.tensor.matmul(
                            ot_ps[0:65, :],
                            v_sb[:, jt, h, :],
                            pts[sub][:, jt, :],
                            start=(jt == 0), stop=(jt == 3))
                    invl = p_small.tile([1, N], f32, tag="invl", name="invl")
                    if RECIP_MODE == "approx_sbuf":
                        lrow = p_small.tile([1, N], f32, tag="lrow",
                                            name="lrow")
                        nc.vector.tensor_copy(lrow[:], ot_ps[64:65, :])
                        nc.vector.reciprocal_approx_fast(invl[:], lrow[:])
                    elif RECIP_MODE == "approx_psum":
                        nc.vector.reciprocal_approx_fast(invl[:], ot_ps[64:65, :])
                    else:
                        nc.vector.reciprocal(invl[:], ot_ps[64:65, :])
                    if BCAST_MODE == "gpsimd":
                        bc_sb = p_small.tile([64, N], f32, tag="bc_sb",
                                             name="bc_sb")
                        nc.gpsimd.partition_broadcast(bc_sb[:], invl[:])
                    else:
                        invl_bf = p_small.tile([1, N], bf16, tag="invl_bf",
                                               name="invl_bf")
                        nc.vector.tensor_copy(invl_bf[:], invl[:])
                        bc_ps = ps_bc.tile([64, N], f32, tag="bc", name="bc_ps")
                        nc.tensor.matmul(
                            bc_ps[:],
                            ones_bc[:],
                            invl_bf[:],
                            start=True, stop=True)
                        bc_sb = p_small.tile([64, N], f32, tag="bc_sb",
                                             name="bc_sb")
                        nc.vector.tensor_copy(bc_sb[:], bc_ps[:])
                    nc.vector.tensor_mul(
                        oT[sub * 64:(sub + 1) * 64, pair, :],
                        ot_ps[0:64, :], bc_sb[:])

            for step in range(5):
                if step < 4:
                    emit_st(step)
                if step >= 1:
                    emit_pv(step - 1)

            # output projection: out[n_tile, c] = sum_t OT[t][:, nslice].T @ wo[t]
            out_sb = p_out.tile([128, 4, C], f32, tag="out")
            for nt in range(4):
                f_ps = ps_proj.tile([128, C], f32, tag="proj")
                for t in range(4):
                    nc.tensor.matmul(
                        f_ps[:],
                        oT[:, t, nt * 128:(nt + 1) * 128],
                        wo_sb[:, t, :],
                        start=(t == 0), stop=(t == 3))
                nc.vector.tensor_copy(out_sb[:, nt, :], f_ps[:])
            nc.sync.dma_start(
                out=out_ext[b].rearrange("(nt p) c -> p nt c", p=128),
                in_=out_sb[:])

    nc.compile()
    return nc


def _get_nc():
    if "nc" not in _cache:
        _cache["nc"] = _build()
    return _cache["nc"]


def kernel(x, pos_bias=None, w_qkv=None, w_out=None, **_ignored):
    from concourse.bass_utils import run_bass_kernel_spmd

    nc = _get_nc()
    xf = np.ascontiguousarray(np.asarray(x, dtype=np.float32).reshape(B * M, N, C))
    wq = np.ascontiguousarray(np.asarray(w_qkv, dtype=np.float32))
    wo = np.ascontiguousarray(np.asarray(w_out, dtype=np.float32))
    in_maps = [
        {"x": xf[i * BPC:(i + 1) * BPC], "w_qkv": wq, "w_out": wo}
        for i in range(NCORES)
    ]
    res = run_bass_kernel_spmd(
        nc, in_maps, core_ids=list(range(NCORES)),
        trace=bool(_cache.get("trace", False)))
    _cache["last_result"] = res
    out = np.concatenate([res.results[i]["out"] for i in range(NCORES)], axis=0)
    return out.reshape(B, M, N, C).astype(np.float32)


# revision 32
# speedup vs baseline: 1.6042x; 1.6042x over previous
"""Trainium2 Bass kernel for nn_Attention (dense_transformer).

Math (per fused-batch element, 32 total = b*m):
    qkv = x @ w_qkv ; split q,k,v into 8 heads of 64
    sim = (q/8) @ k^T  (+ pos_bias term that is constant along the softmax
                        axis -> provably no effect on softmax output, dropped)
    attn = softmax(sim); out = (attn @ v) heads-concat @ w_out

Sharding: pure data-parallel over the fused (b*m)=32 axis -> 4 elements
per core on 8 cores, no collectives. Weights replicated.

Kernel strategy (per core, all-transposed dataflow, bf16 matmuls):
    xT   = PE-transpose(x)                        [c, n]
    qT,kT (pair-stacked) = W_qk^T @ xT            [e_slice, n]  (psum f32)
    V    = xT-slices @ W_v                        [n, e_v] natural layout,
           stored interleaved [n, h, 65] with a ones column per head
    S^T  = kT_h^T-slice @ qT_h                    [j, i] per head
    P^T  = exp(0.125 * S^T)  (no max subtraction: |logits| <= ~8)
    outT_h (rows 0..63) + L_h (row 64) = V1_h^T @ P^T   (ones-column trick)
    OT   = outT_h * broadcast(1/L_h)  (K=1 matmul broadcast + DVE mul)
    out  = OT-slices^T @ w_out        [n, c] -> DMA out
"""

import os
import sys

for _p in ("/root/.axon_site/_ro/trn_rl_repo", "/opt/trn_rl_repo"):
    if os.path.isdir(_p) and _p not in sys.path:
        sys.path.append(_p)

import numpy as np

# ---- problem constants (hardcoded per spec) ----
B, M, N, C = 4, 8, 512, 512
HEADS, DHEAD = 8, 64
E3 = 3 * 512
NCORES = 8
BPC = (B * M) // NCORES  # batch elements per core = 4
BCAST_MODE = "gpsimd"  # "gpsimd" | "pe"
RECIP_MODE = "approx_sbuf"  # "exact" | "approx_sbuf" | "approx_psum"
TR_MODE = "pe"  # "dma" (xbar transpose) | "pe" (tensor-engine transpose)
ACT_COPIES = False  # offload some psum->sbuf copies to the Scalar engine

_cache = {}


def _build():
    import concourse.bass as bass
    import concourse.mybir as mybir
    import concourse.tile as tile
    from concourse import bacc
    from concourse.masks import make_identity

    f32 = mybir.dt.float32
    bf16 = mybir.dt.bfloat16
    f32r = mybir.dt.float32r
    EXP = mybir.ActivationFunctionType.Exp

    nc = bacc.Bacc("TRN2", target_bir_lowering=False, debug=False,
                   num_devices=NCORES)

    x_ext = nc.declare_dram_parameter("x", [BPC, N, C], f32, isOutput=False)
    wq_ext = nc.declare_dram_parameter("w_qkv", [C, E3], f32, isOutput=False)
    wo_ext = nc.declare_dram_parameter("w_out", [512, 512], f32, isOutput=False)
    out_ext = nc.declare_dram_parameter("out", [BPC, N, C], f32, isOutput=True)

    from contextlib import ExitStack

    with tile.TileContext(nc) as tc, ExitStack() as ctx:
        # ---------------- pools ----------------
        p_const = ctx.enter_context(tc.tile_pool(name="const", bufs=1))
        p_stage = ctx.enter_context(tc.tile_pool(name="stage", bufs=1))
        p_x = ctx.enter_context(tc.tile_pool(name="x", bufs=2))
        p_xT = ctx.enter_context(tc.tile_pool(name="xT", bufs=2))
        p_qk = ctx.enter_context(tc.tile_pool(name="qk", bufs=2))
        p_v = ctx.enter_context(tc.tile_pool(name="v", bufs=2))
        p_pt = ctx.enter_context(tc.tile_pool(name="pt", bufs=3))
        p_oT = ctx.enter_context(tc.tile_pool(name="oT", bufs=2))
        p_out = ctx.enter_context(tc.tile_pool(name="out", bufs=2))
        p_small = ctx.enter_context(tc.tile_pool(name="small", bufs=4))

        # tr and st share one 4-deep pool (same tag) so STs can run 4 ahead
        # of the ACT exps; 4 + 2 + 2 = 8 PSUM banks exactly
        ps_st = ctx.enter_context(tc.tile_pool(name="ps_st", bufs=4, space="PSUM"))
        ps_tr = ps_st
        ps_proj = ctx.enter_context(tc.tile_pool(name="ps_proj", bufs=2, space="PSUM"))
        ps_ot = ctx.enter_context(tc.tile_pool(name="ps_ot", bufs=2, space="PSUM"))
        if BCAST_MODE != "gpsimd":
            ps_bc = ctx.enter_context(
                tc.tile_pool(name="ps_bc", bufs=1, space="PSUM"))

        # ---------------- constants ----------------
        # emission order matters for the gpsimd FIFO at startup: identity
        # (tiny, needed by batch-0 transposes), then batch-0's x chunks,
        # then the weights -- so the PE can start transposing ASAP.
        ident = p_const.tile([128, 128], bf16)
        make_identity(nc, ident[:])
        ones_bc = p_const.tile([1, 64], bf16)
        nc.vector.memset(ones_bc[:], 1.0)

        x0_sb = p_x.tile([128, 4, C], bf16, tag="x", name="x_sb")
        x0_r = x_ext[0].rearrange("(nt p) c -> p nt c", p=128)
        for ct in range(4):
            nc.gpsimd.dma_start(out=x0_sb[:, :, ct * 128:(ct + 1) * 128],
                                in_=x0_r[:, :, ct * 128:(ct + 1) * 128])

        # weights: gpsimd SWDGE cast-DMAs straight to bf16 (no staging, no
        # DVE work); chunked so the DMA queues work in parallel
        wq_sb = p_const.tile([128, 4, E3], bf16)
        wq_r = wq_ext.ap().rearrange("(ct p) e -> p ct e", p=128)
        for ct in range(4):
            nc.gpsimd.dma_start(out=wq_sb[:, ct, :], in_=wq_r[:, ct, :])
        wo_sb = p_const.tile([128, 4, 512], bf16)
        nc.gpsimd.dma_start(
            out=wo_sb[:],
            in_=wo_ext.ap().rearrange("(t p) c -> p t c", p=128))

        # ---------------- per-batch stage emitters ----------------
        def stage_x(b):
            """x [512,512] f32 -> SBUF bf16 (SWDGE cast DMA on idle gpsimd
            queues; keeps the PE all-bf16 so FWL weight loads stay on)."""
            if b == 0:
                return x0_sb
            x_sb = p_x.tile([128, 4, C], bf16, tag="x", name="x_sb")
            nc.gpsimd.dma_start(
                out=x_sb[:],
                in_=x_ext[b].rearrange("(nt p) c -> p nt c", p=128))
            return x_sb

        def stage_prep(b, x_sb):
            """Return (qkT, v_sb, [emission thunks]) for transposes +
            projections of batch b. Thunks are emitted interleaved with the
            previous batch's attention so the PE FIFO stays dense."""
            xT = p_xT.tile([128, 4, N], bf16, tag="xT", name="xT")
            qkT = p_qk.tile([128, 8, N], bf16, tag="qkT", name="qkT")
            v_sb = p_v.tile([128, 4, 8, 65], bf16, tag="v", name="v_sb")
            thunks = []

            def tr(ct):
                if TR_MODE == "dma":
                    # xbar DMA transpose, SBUF->SBUF bf16; no PE, no DVE
                    for nt in range(4):
                        nc.sync.dma_start(
                            out=xT[:, ct, nt * 128:(nt + 1) * 128],
                            in_=x_sb[:, nt, ct * 128:(ct + 1) * 128],
                            transpose=True)
                else:
                    tr_ps = ps_tr.tile([128, 512], bf16, tag="st",
                                       name="tr_ps")
                    for nt in range(4):
                        nc.tensor.transpose(
                            tr_ps[:, nt * 128:(nt + 1) * 128],
                            x_sb[:, nt, ct * 128:(ct + 1) * 128], ident[:])
                    nc.vector.tensor_copy(xT[:, ct, :], tr_ps[:])

            def proj_qk(s):
                pr_ps = ps_proj.tile([128, N], f32, tag="proj", name="pr_ps")
                for ct in range(4):
                    nc.tensor.matmul(
                        pr_ps[:],
                        wq_sb[:, ct, s * 128:(s + 1) * 128],
                        xT[:, ct, :],
                        start=(ct == 0), stop=(ct == 3))
                nc.vector.tensor_copy(qkT[:, s, :], pr_ps[:])

            def v_ones():
                nc.vector.memset(v_sb[:, :, :, 64:65], 1.0)

            def proj_v(nt):
                pv_ps = ps_proj.tile([128, N], f32, tag="proj", name="pv_ps")
                for ct in range(4):
                    nc.tensor.matmul(
                        pv_ps[:],
                        xT[:, ct, nt * 128:(nt + 1) * 128],
                        wq_sb[:, ct, 1024:1536],
                        start=(ct == 0), stop=(ct == 3))
                nc.vector.tensor_copy(
                    v_sb[:, nt, :, 0:64],
                    pv_ps[:].rearrange("p (h d) -> p h d", d=64))

            for ct in range(4):
                thunks.append(lambda ct=ct: tr(ct))
            thunks.append(v_ones)
            for s in range(8):
                thunks.append(lambda s=s: proj_qk(s))
            for nt in range(4):
                thunks.append(lambda nt=nt: proj_v(nt))
            return qkT, v_sb, thunks

        def emit_st(qkT, pts_by_pair, pair):
            pts = [p_pt.tile([128, 4, N], bf16, tag=f"pt{sub}",
                             name=f"pt{sub}")
                   for sub in range(2)]
            pts_by_pair[pair] = pts
            for jt in range(4):
                for sub in range(2):
                    lo, hi = sub * 64, (sub + 1) * 64
                    st_ps = ps_st.tile([128, N], f32, tag="st", name="st_ps")
                    nc.tensor.matmul(
                        st_ps[:],
                        qkT[lo:hi, 4 + pair, jt * 128:(jt + 1) * 128],
                        qkT[lo:hi, pair, :],
                        start=True, stop=True)
                    nc.scalar.activation(
                        pts[sub][:, jt, :], st_ps[:], EXP,
                        scale=float(DHEAD) ** -0.5)

        def emit_pv(v_sb, oT, pts_by_pair, pair):
            pts = pts_by_pair.pop(pair)
            for sub in range(2):
                h = 2 * pair + sub
                ot_ps = ps_ot.tile([128, N], f32, tag="ot", name="ot_ps")
                for jt in range(4):
                    nc.tensor.matmul(
                        ot_ps[0:65, :],
                        v_sb[:, jt, h, :],
                        pts[sub][:, jt, :],
                        start=(jt == 0), stop=(jt == 3))
                lrow = p_small.tile([1, N], f32, tag="lrow", name="lrow")
                nc.vector.tensor_copy(lrow[:], ot_ps[64:65, :])
                invl = p_small.tile([1, N], f32, tag="invl", name="invl")
                nc.vector.reciprocal_approx_fast(invl[:], lrow[:])
                bc_sb = p_small.tile([64, N], f32, tag="bc_sb", name="bc_sb")
                nc.gpsimd.partition_broadcast(bc_sb[:], invl[:])
                nc.vector.tensor_mul(
                    oT[sub * 64:(sub + 1) * 64, pair, :],
                    ot_ps[0:64, :], bc_sb[:])

        def stage_attn_steps(qkT, v_sb):
            """Return (oT, [5 step thunks]): STs/exps run one pair ahead of
            PVs so the PE never head-of-line blocks on ACT."""
            oT = p_oT.tile([128, 4, N], bf16, tag="oT", name="oT")
            pts_by_pair = {}
            steps = []
            for step in range(5):
                def thunk(step=step):
                    if step < 4:
                        emit_st(qkT, pts_by_pair, step)
                    if step >= 1:
                        emit_pv(v_sb, oT, pts_by_pair, step - 1)
                steps.append(thunk)
            return oT, steps

        def stage_out(b, oT):
            out_sb = p_out.tile([128, 4, C], f32, tag="out", name="out_sb")
            for nt in range(4):
                f_ps = ps_proj.tile([128, C], f32, tag="proj", name="f_ps")
                for t in range(4):
                    nc.tensor.matmul(
                        f_ps[:],
                        oT[:, t, nt * 128:(nt + 1) * 128],
                        wo_sb[:, t, :],
                        start=(t == 0), stop=(t == 3))
                if ACT_COPIES:
                    nc.scalar.copy(out_sb[:, nt, :], f_ps[:])
                else:
                    nc.vector.tensor_copy(out_sb[:, nt, :], f_ps[:])
            nc.sync.dma_start(
                out=out_ext[b].rearrange("(nt p) c -> p nt c", p=128),
                in_=out_sb[:])

        # ---------------- cross-batch pipeline ----------------
        # While batch b's attention (ACT-bound) runs, batch b+1's transposes
        # and projections (PE-bound) are interleaved into the engine FIFOs,
        # and batch b-1's output projection fills early pair-phase bubbles.
        x_sb = stage_x(0)
        qkT, v_sb, prep_thunks = stage_prep(0, x_sb)
        for t in prep_thunks:
            t()
        pending_out = None  # (b, oT) whose out-projection is deferred
        for b in range(BPC):
            oT, attn_steps = stage_attn_steps(qkT, v_sb)
            next_thunks = []
            if b + 1 < BPC:
                x_next = stage_x(b + 1)
                qkT_n, v_n, next_thunks = stage_prep(b + 1, x_next)
            # interleave: each attention step followed by a few prep thunks
            ni = 0
            for si, step in enumerate(attn_steps):
                step()
                if si == 1 and pending_out is not None:
                    stage_out(*pending_out)
                    pending_out = None
                want = (si + 1) * len(next_thunks) // len(attn_steps)
                while ni < want:
                    next_thunks[ni]()
                    ni += 1
            pending_out = (b, oT)
            if b + 1 < BPC:
                qkT, v_sb = qkT_n, v_n
        stage_out(*pending_out)

    nc.compile()
    return nc


def _get_nc():
    if "nc" not in _cache:
        _cache["nc"] = _build()
    return _cache["nc"]


def kernel(x, pos_bias=None, w_qkv=None, w_out=None, **_ignored):
    from concourse.bass_utils import run_bass_kernel_spmd

    nc = _get_nc()
    xf = np.ascontiguousarray(np.asarray(x, dtype=np.float32).reshape(B * M, N, C))
    wq = np.ascontiguousarray(np.asarray(w_qkv, dtype=np.float32))
    wo = np.ascontiguousarray(np.asarray(w_out, dtype=np.float32))
    in_maps = [
        {"x": xf[i * BPC:(i + 1) * BPC], "w_qkv": wq, "w_out": wo}
        for i in range(NCORES)
    ]
    res = run_bass_kernel_spmd(
        nc, in_maps, core_ids=list(range(NCORES)),
        trace=bool(_cache.get("trace", False)))
    _cache["last_result"] = res
    out = np.concatenate([res.results[i]["out"] for i in range(NCORES)], axis=0)
    return out.reshape(B, M, N, C).astype(np.float32)
